# revision 35
# baseline (speedup 1.0000x reference)
"""Block-causal (block=64) MHA + qkv/out projections on 8 NeuronCores.

Sharding: 8 cores = 2 batches x 4 head-groups (4 heads each).
Per core: qkv projection for its heads, block-causal attention for 4 heads
(processed as 2 head-pairs packed across the 128 partitions), partial output
projection over its 256 channels. Host sums the 4 partials per batch + bias.

On-chip layout is feature-major (transposed): scores are computed transposed
(S^T[k, q] = k . q) so no on-chip transposes are needed anywhere. exp runs on
ScalarE straight out of PSUM.

Matmuls run in float32r (full PE rate). f32r rules honored throughout: every
matmul-operand tile is f32r-typed (producers round on write; memset can't
write f32r, so constants stage through f32 + tensor_copy), and matmul outputs
must start at PSUM partition 0.

The softmax denominator rides in the PV matmul for free: the V tiles are laid
out [v0 | 1 | v1 | v2 | 1 | v3] (384 cols) so each head h has a contiguous
128-col [v_h | ones] / [ones | v_h] stationary window; one full 128x128
matmul per (key tile, head) then yields PV rows on one partition half and the
replicated key-sums on the other. Normalization: reciprocal_approx_fast reads
the sum rows from PSUM, a small SBUF->SBUF DMA shifts the reciprocals onto
the PV lanes, and two tensor_muls write the pair-packed [128, 512] attention
tile, which feeds a 2-chain K=128 output projection.

The diagonal 128-key tiles are one matmul over queries [q0:512) into
persistent per-(j, half) p tiles whose disallowed corner (keys 64:128 x
queries [q0:q0+64)) was zeroed once at startup; the exps write around the
corner, so block-causality costs no per-iteration masking.
"""

import numpy as np

import concourse.bass as bass
import concourse.tile as tile
from concourse import bacc
from concourse import mybir

B, N, C = 2, 2048, 1024
H, HD = 16, 64
HPC = 4  # heads per core
CSL = HPC * HD  # 256 channel slice per core
QKW = 2 * CSL  # 512: q then k output channels
NCORES = 8
QBLK = 512
NQB = N // QBLK  # 4
NT = N // 128  # 16 seq tiles of 128
SCALE = HD**-0.5
F32 = mybir.dt.float32
F32R = mybir.dt.float32r
BF16 = mybir.dt.bfloat16
MMDT = BF16  # matmul-operand dtype: bf16 runs the PE at full rate


def build_nc():
    nc = bacc.Bacc("TRN2", target_bir_lowering=False, debug=False, num_devices=NCORES)

    # consolidated inputs: one wide row-major tensor each so the whole
    # input set loads in a handful of big DMAs. x is nb-major: 4096-col
    # blocks of [ct0|ct1|...|ct7] 512-col chunks, so the first block is
    # exactly what the first (nb-major) projection chains consume.
    xT_d = nc.dram_tensor("xT", [4, 128, 8 * QBLK], MMDT, kind="ExternalInput")
    wqk_d = nc.dram_tensor("wqkT", [1, 128, 8 * QKW], MMDT, kind="ExternalInput")
    wv_d = nc.dram_tensor("wvT", [1, 128, 8 * CSL], MMDT, kind="ExternalInput")
    wp_d = nc.dram_tensor("wpT", [1, 128, 2 * C], MMDT, kind="ExternalInput")
    y_d = nc.dram_tensor("y", [N, C], F32, kind="ExternalOutput")

    with tile.TileContext(nc) as tc:
        with (
            tc.tile_pool(name="persist", bufs=1) as persist,
            tc.tile_pool(name="pt", bufs=3) as pt_pool,
            tc.tile_pool(name="rc", bufs=2) as rc_pool,
            tc.tile_pool(name="att", bufs=2) as at_pool,
            tc.tile_pool(name="yout", bufs=2) as y_pool,
            tc.tile_pool(name="psmm", bufs=2, space="PSUM") as ps_mm,
            tc.tile_pool(name="pssc", bufs=2, space="PSUM") as ps_sc,
            tc.tile_pool(name="psacc", bufs=1, space="PSUM") as ps_acc,
        ):
            # ---- load inputs: 7 big DMAs, issued in consumption order ----
            x_all = persist.tile([128, N * 8], MMDT, tag="x_all", name="x_all")
            wqk_all = persist.tile([128, 8 * QKW], MMDT, tag="wqk_all", name="wqk_all")
            wv_all = persist.tile([128, 8 * CSL], MMDT, tag="wv_all", name="wv_all")
            wp_all = persist.tile([128, 2 * C], MMDT, tag="wp_all", name="wp_all")
            # first-needed data (wqk + x block 0) split into quarters across
            # three issue queues so the first projection chain starts ASAP
            dq = [nc.sync, nc.gpsimd, nc.scalar]
            qtr = 2 * QKW
            for i in range(4):
                dq[i % 3].dma_start(
                    out=wqk_all[:, i * qtr : (i + 1) * qtr],
                    in_=wqk_d[0][:, i * qtr : (i + 1) * qtr],
                )
            xq = 2 * QBLK
            for i in range(4):
                dq[(i + 1) % 3].dma_start(
                    out=x_all[:, i * xq : (i + 1) * xq], in_=xT_d[0][:, i * xq : (i + 1) * xq]
                )
            half = 4 * QBLK
            for nb in range(1, NQB):
                c0 = nb * 8 * QBLK
                nc.sync.dma_start(
                    out=x_all[:, c0 : c0 + half], in_=xT_d[nb][:, 0:half]
                )
                nc.gpsimd.dma_start(
                    out=x_all[:, c0 + half : c0 + 8 * QBLK], in_=xT_d[nb][:, half:]
                )
            nc.scalar.dma_start(out=wv_all, in_=wv_d[0])
            nc.scalar.dma_start(out=wp_all, in_=wp_d[0])

            # slice views matching the old per-ct tile layout
            def x_at(ct, nb, off, n):
                # columns [off, off+n) of seq block nb of channel group ct
                return x_all[:, nb * 8 * QBLK + ct * QBLK + off : nb * 8 * QBLK + ct * QBLK + off + n]

            wqks = [wqk_all[:, i * QKW : (i + 1) * QKW] for i in range(8)]
            wvs = [wv_all[:, i * CSL : (i + 1) * CSL] for i in range(8)]
            wp2 = [wp_all[:, i * C : (i + 1) * C] for i in range(2)]

            # memset can't write f32r (ISA check); stage via f32 + rounding copy
            ones_t = persist.tile([128, 128], MMDT, tag="ones")
            ones_f = persist.tile([128, 128], F32, tag="ones_f")
            nc.vector.memset(ones_f, 1.0)
            nc.vector.tensor_copy(out=ones_t, in_=ones_f)
            zero_r = persist.tile([128, 64], MMDT, tag="zero_r")
            zero_f = persist.tile([128, 64], F32, tag="zero_f")
            nc.vector.memset(zero_f, 0.0)
            nc.vector.tensor_copy(out=zero_r, in_=zero_f)

            # persistent diagonal p tiles, one per (j, head-half). Their
            # disallowed corner (keys 64:128 x queries [128j, 128j+64)) is
            # zeroed ONCE here; the diag exps write around it, so PV/sum can
            # read the full [*, q0:512) range with the corner always zero.
            pd = {}
            for j in range(4):
                for hh in range(2):
                    t = persist.tile(
                        [128, QBLK], MMDT, tag=f"pd{j}{hh}", name=f"pd{j}{hh}"
                    )
                    pd[j, hh] = t
                    nc.gpsimd.tensor_copy(
                        out=t[64:128, 128 * j : 128 * j + 64], in_=zero_r[64:128, :]
                    )

            # ---- phase 1: q/k projection, transposed outputs ----
            # k tiles kT[pair]: partitions 0:64 = even head dims, 64:128 =
            # odd. q goes into ZERO-PADDED per-head tiles (head dims on its
            # partition half, zeros on the other) so every score matmul has
            # full K=128 contraction AND the A/B pair shares one stationary.
            kT = [persist.tile([128, N], MMDT, tag=f"k{t}", name=f"kT{t}") for t in range(2)]
            qp = {}
            for pair in range(2):
                for hh in range(2):
                    t = persist.tile([128, N], MMDT, tag=f"qp{pair}{hh}", name=f"qp{pair}{hh}")
                    qp[pair, hh] = t
                    z0 = 64 if hh == 0 else 0  # zero the OTHER half
                    nc.vector.memset(t[z0 : z0 + 64, :], 0.0)
            def emit_phase1(nb):
                for dt_ in range(4):
                    ps = ps_mm.tile([128, QBLK], F32, tag="mm", name="ps1")
                    for ct in range(8):
                        nc.tensor.matmul(
                            ps,
                            lhsT=wqks[ct][:, dt_ * 128 : (dt_ + 1) * 128],
                            rhs=x_at(ct, nb, 0, QBLK),
                            start=(ct == 0),
                            stop=(ct == 7),
                        )
                    nbs = slice(nb * QBLK, (nb + 1) * QBLK)
                    if dt_ >= 2:  # k heads
                        nc.vector.tensor_copy(out=kT[dt_ - 2][:, nbs], in_=ps)
                    else:  # q heads: split into the padded per-head tiles
                        nc.vector.tensor_copy(
                            out=qp[dt_, 0][0:64, nbs], in_=ps[0:64, :]
                        )
                        nc.vector.tensor_copy(
                            out=qp[dt_, 1][64:128, nbs], in_=ps[64:128, :]
                        )

            # ---- phase 2: v projection into [v0 | 1 | v1 | v2 | 1 | v3] ----
            # head h's PV stationary is the 128-col window starting at 64*h
            # offset... concretely: head 0 -> cols 0:128 ([v0|1]), head 1 ->
            # cols 64:192 ([1|v1]), head 2 -> 192:320 ([v2|1]), head 3 ->
            # 256:384 ([1|v3]).
            v65 = [persist.tile([128, 384], MMDT, tag=f"v{t}", name=f"v{t}") for t in range(NT)]
            for nt in range(NT):
                nc.gpsimd.tensor_copy(out=v65[nt][:, 64:128], in_=ones_t[:, 0:64])
                nc.gpsimd.tensor_copy(out=v65[nt][:, 256:320], in_=ones_t[:, 0:64])

            def emit_phase2(nt):
                ps = ps_mm.tile([128, CSL], F32, tag="mm", name="ps2")
                for ct in range(8):
                    nc.tensor.matmul(
                        ps,
                        lhsT=x_at(ct, nt // 4, (nt % 4) * 128, 128),
                        rhs=wvs[ct],
                        start=(ct == 0),
                        stop=(ct == 7),
                    )
                nc.vector.tensor_copy(out=v65[nt][:, 0:64], in_=ps[:, 0:64])
                nc.vector.tensor_copy(out=v65[nt][:, 128:256], in_=ps[:, 64:192])
                nc.vector.tensor_copy(out=v65[nt][:, 320:384], in_=ps[:, 192:256])

            # stationary windows: (even head A, odd head B) per pair
            def vwin(kt, pair, hh):
                base = pair * 192 + hh * 64
                return v65[kt][:, base : base + 128]

            # ---- phase 3+4: attention (per 512-query block), then out-proj ----
            def emit_outproj(qi, a2, which):
                # output projection for 2 of query block qi's 4 row tiles
                for nt in [4 * qi + w for w in which]:
                    ysb = y_pool.tile([128, C], F32, tag="y", name=f"ysb{nt}")
                    ntl = (nt - 4 * qi) * 128
                    for cb in range(2):
                        psy = ps_mm.tile([128, QBLK], F32, tag="mm", name="psy")
                        for pr in range(2):
                            nc.tensor.matmul(
                                psy,
                                lhsT=a2[pr][:, ntl : ntl + 128],
                                rhs=wp2[pr][:, cb * QBLK : (cb + 1) * QBLK],
                                start=(pr == 0),
                                stop=(pr == 1),
                            )
                        nc.vector.tensor_copy(
                            out=ysb[:, cb * QBLK : (cb + 1) * QBLK], in_=psy
                        )
                    # gpsimd queue: sync carries the normalize-shift DMAs
                    nc.gpsimd.dma_start(
                        out=y_d[nt * 128 : (nt + 1) * 128, :], in_=ysb
                    )

            pending = None  # (qi, a2) whose out-proj is deferred one block
            for qi in range(NQB):
                # projections for this block: attention qi only needs q/k
                # columns and v tiles through block qi, so projection work
                # interleaves with (and PE-feeds during) earlier attention
                emit_phase1(qi)
                for nt in range(4 * qi, 4 * qi + 4):
                    emit_phase2(nt)
                a2 = [
                    at_pool.tile([128, QBLK], MMDT, tag=f"a{p}", name=f"a{p}_{qi}")
                    for p in range(2)
                ]
                for pair in range(2):
                    qtA = qp[pair, 0]
                    qtB = qp[pair, 1]
                    kt_t = kT[pair]
                    qs = slice(qi * QBLK, (qi + 1) * QBLK)

                    # one PSUM bank per head: PV rows on one partition half,
                    # replicated key-sums on the other (from the ones block
                    # of the stationary). A: PV 0:64 / sums 64:128; B: sums
                    # 0:64 / PV 64:128.
                    at_bA = ps_acc.tile([128, QBLK], F32, tag="atA", name="at_bA")
                    at_bB = ps_acc.tile([128, QBLK], F32, tag="atB", name="at_bB")

                    n_reg = 4 * qi
                    n_per_range = n_reg + 4
                    at_A, at_B = [0], [0]

                    def fl(cnt, total=n_per_range):
                        i = cnt[0]
                        cnt[0] += 1
                        return dict(start=(i == 0), stop=(i == total - 1))

                    # work items: rect key tiles then diagonal tiles
                    items = [("r", kt) for kt in range(n_reg)]
                    items += [("d", j) for j in range(4)]

                    def emit_scores(item):
                        """score matmuls + exps for one key tile; returns the
                        p tiles + query range for the later PV matmuls."""
                        kind, idx = item
                        psA = ps_sc.tile([128, QBLK], F32, tag="sA", name="psA")
                        psB = ps_sc.tile([128, QBLK], F32, tag="sB", name="psB")
                        if kind == "r":
                            kt, q0 = idx, 0
                            ks = slice(kt * 128, (kt + 1) * 128)
                            nc.tensor.matmul(
                                psA, lhsT=kt_t[:, ks], rhs=qtA[:, qs],
                                start=True, stop=True,
                            )
                            nc.tensor.matmul(
                                psB, lhsT=kt_t[:, ks], rhs=qtB[:, qs],
                                start=True, stop=True,
                            )
                            pA = pt_pool.tile([128, QBLK], MMDT, tag="pA", name="pA")
                            pB = pt_pool.tile([128, QBLK], MMDT, tag="pB", name="pB")
                            nc.scalar.activation(
                                out=pA, in_=psA,
                                func=mybir.ActivationFunctionType.Exp, scale=SCALE,
                            )
                            nc.scalar.activation(
                                out=pB, in_=psB,
                                func=mybir.ActivationFunctionType.Exp, scale=SCALE,
                            )
                        else:
                            j = idx
                            kt = 4 * qi + j
                            q0 = 128 * j
                            q1 = q0 + 64
                            ksl = slice(kt * 128, (kt + 1) * 128)
                            qsl = slice(qi * QBLK + q0, (qi + 1) * QBLK)
                            pA = pd[j, 0]
                            pB = pd[j, 1]
                            for q_t, ps_s, p_s in ((qtA, psA, pA), (qtB, psB, pB)):
                                nc.tensor.matmul(
                                    ps_s[:, q0:QBLK], lhsT=kt_t[:, ksl],
                                    rhs=q_t[:, qsl], start=True, stop=True,
                                )
                                # one exp (ScalarE is the attention-phase
                                # bottleneck), then GpSimd re-zeroes the
                                # disallowed corner it overwrote
                                nc.scalar.activation(
                                    out=p_s[:, q0:QBLK], in_=ps_s[:, q0:QBLK],
                                    func=mybir.ActivationFunctionType.Exp,
                                    scale=SCALE,
                                )
                                nc.gpsimd.tensor_copy(
                                    out=p_s[64:128, q0:q1], in_=zero_r[64:128, :]
                                )
                        return kt, q0, pA, pB

                    def emit_pv(staged):
                        kt, q0, pA, pB = staged
                        nc.tensor.matmul(
                            at_bA[:, q0:QBLK], lhsT=vwin(kt, pair, 0),
                            rhs=pA[:, q0:QBLK], **fl(at_A),
                        )
                        nc.tensor.matmul(
                            at_bB[:, q0:QBLK], lhsT=vwin(kt, pair, 1),
                            rhs=pB[:, q0:QBLK], **fl(at_B),
                        )

                    # software-pipelined, lookahead 2: scores run two key
                    # tiles ahead of PV so the exp latency hides under score
                    # work. Half of the previous query block's out-proj is
                    # emitted right after each pair's score prologue - those
                    # 4 matmuls don't touch the accumulator banks, so they
                    # cover the banks' normalize-chain WAR latency.
                    stagedq = [emit_scores(it) for it in items[:2]]
                    if pending:
                        emit_outproj(*pending, (0, 1) if pair == 0 else (2, 3))
                        if pair == 1:
                            pending = None
                    for i in range(len(items)):
                        if i + 2 < len(items):
                            stagedq.append(emit_scores(items[i + 2]))
                        emit_pv(stagedq.pop(0))

                    # normalize: copy/reciprocal out of PSUM, DMA shifts the
                    # values onto the PV lanes, fast reciprocal for A's half
                    # (custom-DVE ops misread PSUM base 64, so A's sums are
                    # copied out and reciprocated after the shift)
                    rec = rc_pool.tile([128, QBLK], F32, tag="rec")
                    rsh = rc_pool.tile([128, QBLK], F32, tag="rsh")
                    rcf = rc_pool.tile([128, QBLK], F32, tag="rcf")
                    nc.vector.tensor_copy(
                        out=rec[64:128, :], in_=at_bA[64:128, :]
                    )
                    nc.vector.reciprocal_approx_fast(
                        out=rec[0:64, :], in_=at_bB[0:64, :]
                    )
                    nc.sync.dma_start(out=rsh[0:64, :], in_=rec[64:128, :])
                    nc.sync.dma_start(out=rsh[64:128, :], in_=rec[0:64, :])
                    nc.vector.reciprocal_approx_fast(
                        out=rcf[0:64, :], in_=rsh[0:64, :]
                    )
                    nc.vector.tensor_mul(
                        out=a2[pair][0:64, :], in0=at_bA[0:64, :], in1=rcf[0:64, :]
                    )
                    nc.vector.tensor_mul(
                        out=a2[pair][64:128, :], in0=at_bB[64:128, :],
                        in1=rsh[64:128, :],
                    )

                pending = (qi, a2)
            emit_outproj(*pending, (0, 1, 2, 3))

    return nc


def _shard_inputs(x, w_qkv, w_proj):
    import ml_dtypes

    bf16 = ml_dtypes.bfloat16
    x = np.ascontiguousarray(np.asarray(x, dtype=np.float32).astype(bf16))
    w_qkv = np.asarray(w_qkv, dtype=np.float32).astype(bf16)
    w_proj = np.asarray(w_proj, dtype=np.float32).astype(bf16)
    # x_all layout: [nb, partition, ct*512 + q] (nb-major column blocks)
    xT = []
    for b in range(B):
        t = np.ascontiguousarray(x[b].T).reshape(8, 128, NQB, QBLK)
        xT.append(
            np.ascontiguousarray(t.transpose(2, 1, 0, 3)).reshape(NQB, 128, 8 * QBLK)
        )
    in_maps = []
    for c in range(NCORES):
        b, g = divmod(c, 4)
        r0 = 64 * HPC * g  # 256 * g
        wq = w_qkv[r0 : r0 + CSL, :]
        wk = w_qkv[C + r0 : C + r0 + CSL, :]
        wvs = w_qkv[2 * C + r0 : 2 * C + r0 + CSL, :]
        wqkT = np.concatenate([wq, wk], axis=0).T.reshape(8, 128, QKW)
        wqkT = np.ascontiguousarray(wqkT.transpose(1, 0, 2)).reshape(1, 128, 8 * QKW)
        wvT = wvs.T.reshape(8, 128, CSL)
        wvT = np.ascontiguousarray(wvT.transpose(1, 0, 2)).reshape(1, 128, 8 * CSL)
        wpT = w_proj[:, r0 : r0 + CSL].T.reshape(2, 128, C)
        wpT = np.ascontiguousarray(wpT.transpose(1, 0, 2)).reshape(1, 128, 2 * C)
        in_maps.append({"xT": xT[b], "wqkT": wqkT, "wvT": wvT, "wpT": wpT})
    return in_maps


def run(x, w_qkv, w_proj, b_proj, trace=False, **spmd_kwargs):
    from concourse.bass_utils import run_bass_kernel_spmd

    in_maps = _shard_inputs(x, w_qkv, w_proj)
    nc = build_nc()
    nc.finalize()
    res = run_bass_kernel_spmd(
        nc, in_maps, core_ids=list(range(NCORES)), trace=trace, **spmd_kwargs
    )
    y = np.zeros((B, N, C), np.float32)
    for c in range(NCORES):
        y[c // 4] += res.results[c]["y"]
    y += np.asarray(b_proj, dtype=np.float32)[None, None, :]
    return y, res


def kernel(x, w_qkv, w_proj, b_proj):
    y, _ = run(x, w_qkv, w_proj, b_proj, trace=False)
    return y


# revision 37
# speedup vs baseline: 1.0154x; 1.0154x over previous
"""Block-causal (block=64) MHA + qkv/out projections on 8 NeuronCores.

Sharding: 8 cores = 2 batches x 4 head-groups (4 heads each).
Per core: qkv projection for its heads, block-causal attention for 4 heads
(processed as 2 head-pairs packed across the 128 partitions), partial output
projection over its 256 channels. Host sums the 4 partials per batch + bias.

On-chip layout is feature-major (transposed): scores are computed transposed
(S^T[k, q] = k . q) so no on-chip transposes are needed anywhere. exp runs on
ScalarE straight out of PSUM.

Matmuls run in float32r (full PE rate). f32r rules honored throughout: every
matmul-operand tile is f32r-typed (producers round on write; memset can't
write f32r, so constants stage through f32 + tensor_copy), and matmul outputs
must start at PSUM partition 0.

The softmax denominator rides in the PV matmul for free: the V tiles are laid
out [v0 | 1 | v1 | v2 | 1 | v3] (384 cols) so each head h has a contiguous
128-col [v_h | ones] / [ones | v_h] stationary window; one full 128x128
matmul per (key tile, head) then yields PV rows on one partition half and the
replicated key-sums on the other. Normalization: reciprocal_approx_fast reads
the sum rows from PSUM, a small SBUF->SBUF DMA shifts the reciprocals onto
the PV lanes, and two tensor_muls write the pair-packed [128, 512] attention
tile, which feeds a 2-chain K=128 output projection.

The diagonal 128-key tiles are one matmul over queries [q0:512) into
persistent per-(j, half) p tiles whose disallowed corner (keys 64:128 x
queries [q0:q0+64)) was zeroed once at startup; the exps write around the
corner, so block-causality costs no per-iteration masking.
"""

import numpy as np

import concourse.bass as bass
import concourse.tile as tile
from concourse import bacc
from concourse import mybir

B, N, C = 2, 2048, 1024
H, HD = 16, 64
HPC = 4  # heads per core
CSL = HPC * HD  # 256 channel slice per core
QKW = 2 * CSL  # 512: q then k output channels
NCORES = 8
QBLK = 512
NQB = N // QBLK  # 4
NT = N // 128  # 16 seq tiles of 128
SCALE = HD**-0.5
F32 = mybir.dt.float32
F32R = mybir.dt.float32r
BF16 = mybir.dt.bfloat16
MMDT = BF16  # matmul-operand dtype: bf16 runs the PE at full rate


def build_nc():
    nc = bacc.Bacc("TRN2", target_bir_lowering=False, debug=False, num_devices=NCORES)

    # consolidated inputs: one wide row-major tensor each so the whole
    # input set loads in a handful of big DMAs. x is nb-major: 4096-col
    # blocks of [ct0|ct1|...|ct7] 512-col chunks, so the first block is
    # exactly what the first (nb-major) projection chains consume.
    xT_d = nc.dram_tensor("xT", [4, 128, 8 * QBLK], MMDT, kind="ExternalInput")
    wqk_d = nc.dram_tensor("wqkT", [1, 128, 8 * QKW], MMDT, kind="ExternalInput")
    wv_d = nc.dram_tensor("wvT", [1, 128, 8 * CSL], MMDT, kind="ExternalInput")
    wp_d = nc.dram_tensor("wpT", [1, 128, 2 * C], MMDT, kind="ExternalInput")
    y_d = nc.dram_tensor("y", [N, C], F32, kind="ExternalOutput")

    with tile.TileContext(nc) as tc:
        with (
            tc.tile_pool(name="persist", bufs=1) as persist,
            tc.tile_pool(name="pt", bufs=4) as pt_pool,
            tc.tile_pool(name="rc", bufs=2) as rc_pool,
            tc.tile_pool(name="att", bufs=2) as at_pool,
            tc.tile_pool(name="yout", bufs=2) as y_pool,
            tc.tile_pool(name="psmm", bufs=2, space="PSUM") as ps_mm,
            tc.tile_pool(name="pssc", bufs=2, space="PSUM") as ps_sc,
            tc.tile_pool(name="psacc", bufs=1, space="PSUM") as ps_acc,
        ):
            # ---- load inputs: 7 big DMAs, issued in consumption order ----
            x_all = persist.tile([128, N * 8], MMDT, tag="x_all", name="x_all")
            wqk_all = persist.tile([128, 8 * QKW], MMDT, tag="wqk_all", name="wqk_all")
            wv_all = persist.tile([128, 8 * CSL], MMDT, tag="wv_all", name="wv_all")
            wp_all = persist.tile([128, 2 * C], MMDT, tag="wp_all", name="wp_all")
            # first-needed data (wqk + x block 0) split into quarters across
            # three issue queues so the first projection chain starts ASAP
            dq = [nc.sync, nc.gpsimd, nc.scalar]
            qtr = 2 * QKW
            for i in range(4):
                dq[i % 3].dma_start(
                    out=wqk_all[:, i * qtr : (i + 1) * qtr],
                    in_=wqk_d[0][:, i * qtr : (i + 1) * qtr],
                )
            xq = 2 * QBLK
            for i in range(4):
                dq[(i + 1) % 3].dma_start(
                    out=x_all[:, i * xq : (i + 1) * xq], in_=xT_d[0][:, i * xq : (i + 1) * xq]
                )
            half = 4 * QBLK
            for nb in range(1, NQB):
                c0 = nb * 8 * QBLK
                nc.sync.dma_start(
                    out=x_all[:, c0 : c0 + half], in_=xT_d[nb][:, 0:half]
                )
                nc.gpsimd.dma_start(
                    out=x_all[:, c0 + half : c0 + 8 * QBLK], in_=xT_d[nb][:, half:]
                )
            nc.scalar.dma_start(out=wv_all, in_=wv_d[0])
            nc.scalar.dma_start(out=wp_all, in_=wp_d[0])

            # slice views matching the old per-ct tile layout
            def x_at(ct, nb, off, n):
                # columns [off, off+n) of seq block nb of channel group ct
                return x_all[:, nb * 8 * QBLK + ct * QBLK + off : nb * 8 * QBLK + ct * QBLK + off + n]

            wqks = [wqk_all[:, i * QKW : (i + 1) * QKW] for i in range(8)]
            wvs = [wv_all[:, i * CSL : (i + 1) * CSL] for i in range(8)]
            wp2 = [wp_all[:, i * C : (i + 1) * C] for i in range(2)]

            # memset can't write f32r (ISA check); stage via f32 + rounding copy
            ones_t = persist.tile([128, 128], MMDT, tag="ones")
            ones_f = persist.tile([128, 128], F32, tag="ones_f")
            nc.vector.memset(ones_f, 1.0)
            nc.vector.tensor_copy(out=ones_t, in_=ones_f)
            zero_r = persist.tile([128, 64], MMDT, tag="zero_r")
            zero_f = persist.tile([128, 64], F32, tag="zero_f")
            nc.vector.memset(zero_f, 0.0)
            nc.vector.tensor_copy(out=zero_r, in_=zero_f)

            # persistent diagonal p tiles, one per (j, head-half). Their
            # disallowed corner (keys 64:128 x queries [128j, 128j+64)) is
            # zeroed ONCE here; the diag exps write around it, so PV/sum can
            # read the full [*, q0:512) range with the corner always zero.
            pd = {}
            for j in range(4):
                for hh in range(2):
                    t = persist.tile(
                        [128, QBLK], MMDT, tag=f"pd{j}{hh}", name=f"pd{j}{hh}"
                    )
                    pd[j, hh] = t
                    nc.gpsimd.tensor_copy(
                        out=t[64:128, 128 * j : 128 * j + 64], in_=zero_r[64:128, :]
                    )

            # ---- phase 1: q/k projection, transposed outputs ----
            # k tiles kT[pair]: partitions 0:64 = even head dims, 64:128 =
            # odd. q goes into ZERO-PADDED per-head tiles (head dims on its
            # partition half, zeros on the other) so every score matmul has
            # full K=128 contraction AND the A/B pair shares one stationary.
            kT = [persist.tile([128, N], MMDT, tag=f"k{t}", name=f"kT{t}") for t in range(2)]
            qp = {}
            for pair in range(2):
                for hh in range(2):
                    t = persist.tile([128, N], MMDT, tag=f"qp{pair}{hh}", name=f"qp{pair}{hh}")
                    qp[pair, hh] = t
                    z0 = 64 if hh == 0 else 0  # zero the OTHER half
                    nc.vector.memset(t[z0 : z0 + 64, :], 0.0)
            def emit_phase1(nb):
                for dt_ in range(4):
                    ps = ps_mm.tile([128, QBLK], F32, tag="mm", name="ps1")
                    for ct in range(8):
                        nc.tensor.matmul(
                            ps,
                            lhsT=wqks[ct][:, dt_ * 128 : (dt_ + 1) * 128],
                            rhs=x_at(ct, nb, 0, QBLK),
                            start=(ct == 0),
                            stop=(ct == 7),
                        )
                    nbs = slice(nb * QBLK, (nb + 1) * QBLK)
                    if dt_ >= 2:  # k heads
                        nc.vector.tensor_copy(out=kT[dt_ - 2][:, nbs], in_=ps)
                    else:  # q heads: split into the padded per-head tiles
                        nc.vector.tensor_copy(
                            out=qp[dt_, 0][0:64, nbs], in_=ps[0:64, :]
                        )
                        nc.vector.tensor_copy(
                            out=qp[dt_, 1][64:128, nbs], in_=ps[64:128, :]
                        )

            # ---- phase 2: v projection into [v0 | 1 | v1 | v2 | 1 | v3] ----
            # head h's PV stationary is the 128-col window starting at 64*h
            # offset... concretely: head 0 -> cols 0:128 ([v0|1]), head 1 ->
            # cols 64:192 ([1|v1]), head 2 -> 192:320 ([v2|1]), head 3 ->
            # 256:384 ([1|v3]).
            v65 = [persist.tile([128, 384], MMDT, tag=f"v{t}", name=f"v{t}") for t in range(NT)]
            for nt in range(NT):
                nc.gpsimd.tensor_copy(out=v65[nt][:, 64:128], in_=ones_t[:, 0:64])
                nc.gpsimd.tensor_copy(out=v65[nt][:, 256:320], in_=ones_t[:, 0:64])

            def emit_phase2(nt):
                ps = ps_mm.tile([128, CSL], F32, tag="mm", name="ps2")
                for ct in range(8):
                    nc.tensor.matmul(
                        ps,
                        lhsT=x_at(ct, nt // 4, (nt % 4) * 128, 128),
                        rhs=wvs[ct],
                        start=(ct == 0),
                        stop=(ct == 7),
                    )
                nc.vector.tensor_copy(out=v65[nt][:, 0:64], in_=ps[:, 0:64])
                nc.vector.tensor_copy(out=v65[nt][:, 128:256], in_=ps[:, 64:192])
                nc.vector.tensor_copy(out=v65[nt][:, 320:384], in_=ps[:, 192:256])

            # stationary windows: (even head A, odd head B) per pair
            def vwin(kt, pair, hh):
                base = pair * 192 + hh * 64
                return v65[kt][:, base : base + 128]

            # ---- phase 3+4: attention (per 512-query block), then out-proj ----
            def emit_outproj(qi, a2, which):
                # output projection for 2 of query block qi's 4 row tiles
                for nt in [4 * qi + w for w in which]:
                    ysb = y_pool.tile([128, C], F32, tag="y", name=f"ysb{nt}")
                    ntl = (nt - 4 * qi) * 128
                    for cb in range(2):
                        psy = ps_mm.tile([128, QBLK], F32, tag="mm", name="psy")
                        for pr in range(2):
                            nc.tensor.matmul(
                                psy,
                                lhsT=a2[pr][:, ntl : ntl + 128],
                                rhs=wp2[pr][:, cb * QBLK : (cb + 1) * QBLK],
                                start=(pr == 0),
                                stop=(pr == 1),
                            )
                        nc.vector.tensor_copy(
                            out=ysb[:, cb * QBLK : (cb + 1) * QBLK], in_=psy
                        )
                    nc.sync.dma_start(out=y_d[nt * 128 : (nt + 1) * 128, :], in_=ysb)

            pending = None  # (qi, a2) whose out-proj is deferred one block
            for qi in range(NQB):
                # projections for this block: attention qi only needs q/k
                # columns and v tiles through block qi, so projection work
                # interleaves with (and PE-feeds during) earlier attention
                emit_phase1(qi)
                for nt in range(4 * qi, 4 * qi + 4):
                    emit_phase2(nt)
                a2 = [
                    at_pool.tile([128, QBLK], MMDT, tag=f"a{p}", name=f"a{p}_{qi}")
                    for p in range(2)
                ]
                for pair in range(2):
                    qtA = qp[pair, 0]
                    qtB = qp[pair, 1]
                    kt_t = kT[pair]
                    qs = slice(qi * QBLK, (qi + 1) * QBLK)

                    # one PSUM bank per head: PV rows on one partition half,
                    # replicated key-sums on the other (from the ones block
                    # of the stationary). A: PV 0:64 / sums 64:128; B: sums
                    # 0:64 / PV 64:128.
                    at_bA = ps_acc.tile([128, QBLK], F32, tag="atA", name="at_bA")
                    at_bB = ps_acc.tile([128, QBLK], F32, tag="atB", name="at_bB")

                    n_reg = 4 * qi
                    n_per_range = n_reg + 4
                    at_A, at_B = [0], [0]

                    def fl(cnt, total=n_per_range):
                        i = cnt[0]
                        cnt[0] += 1
                        return dict(start=(i == 0), stop=(i == total - 1))

                    # work items: rect key tiles then diagonal tiles
                    items = [("r", kt) for kt in range(n_reg)]
                    items += [("d", j) for j in range(4)]

                    def emit_scores(item):
                        """score matmuls + exps for one key tile; returns the
                        p tiles + query range for the later PV matmuls."""
                        kind, idx = item
                        psA = ps_sc.tile([128, QBLK], F32, tag="sA", name="psA")
                        psB = ps_sc.tile([128, QBLK], F32, tag="sB", name="psB")
                        if kind == "r":
                            kt, q0 = idx, 0
                            ks = slice(kt * 128, (kt + 1) * 128)
                            nc.tensor.matmul(
                                psA, lhsT=kt_t[:, ks], rhs=qtA[:, qs],
                                start=True, stop=True,
                            )
                            nc.tensor.matmul(
                                psB, lhsT=kt_t[:, ks], rhs=qtB[:, qs],
                                start=True, stop=True,
                            )
                            pA = pt_pool.tile([128, QBLK], MMDT, tag="pA", name="pA")
                            pB = pt_pool.tile([128, QBLK], MMDT, tag="pB", name="pB")
                            nc.scalar.activation(
                                out=pA, in_=psA,
                                func=mybir.ActivationFunctionType.Exp, scale=SCALE,
                            )
                            nc.scalar.activation(
                                out=pB, in_=psB,
                                func=mybir.ActivationFunctionType.Exp, scale=SCALE,
                            )
                        else:
                            j = idx
                            kt = 4 * qi + j
                            q0 = 128 * j
                            q1 = q0 + 64
                            ksl = slice(kt * 128, (kt + 1) * 128)
                            qsl = slice(qi * QBLK + q0, (qi + 1) * QBLK)
                            pA = pd[j, 0]
                            pB = pd[j, 1]
                            for q_t, ps_s, p_s in ((qtA, psA, pA), (qtB, psB, pB)):
                                nc.tensor.matmul(
                                    ps_s[:, q0:QBLK], lhsT=kt_t[:, ksl],
                                    rhs=q_t[:, qsl], start=True, stop=True,
                                )
                                # one exp (ScalarE is the attention-phase
                                # bottleneck), then GpSimd re-zeroes the
                                # disallowed corner it overwrote
                                nc.scalar.activation(
                                    out=p_s[:, q0:QBLK], in_=ps_s[:, q0:QBLK],
                                    func=mybir.ActivationFunctionType.Exp,
                                    scale=SCALE,
                                )
                                nc.gpsimd.tensor_copy(
                                    out=p_s[64:128, q0:q1], in_=zero_r[64:128, :]
                                )
                        return kt, q0, pA, pB

                    def emit_pv(staged):
                        kt, q0, pA, pB = staged
                        nc.tensor.matmul(
                            at_bA[:, q0:QBLK], lhsT=vwin(kt, pair, 0),
                            rhs=pA[:, q0:QBLK], **fl(at_A),
                        )
                        nc.tensor.matmul(
                            at_bB[:, q0:QBLK], lhsT=vwin(kt, pair, 1),
                            rhs=pB[:, q0:QBLK], **fl(at_B),
                        )

                    # software-pipelined, lookahead 2: scores run two key
                    # tiles ahead of PV so the exp latency hides under score
                    # work. Half of the previous query block's out-proj is
                    # emitted right after each pair's score prologue - those
                    # 4 matmuls don't touch the accumulator banks, so they
                    # cover the banks' normalize-chain WAR latency.
                    stagedq = [emit_scores(it) for it in items[:3]]
                    if pending:
                        emit_outproj(*pending, (0, 1) if pair == 0 else (2, 3))
                        if pair == 1:
                            pending = None
                    for i in range(len(items)):
                        if i + 3 < len(items):
                            stagedq.append(emit_scores(items[i + 3]))
                        emit_pv(stagedq.pop(0))

                    # normalize: copy/reciprocal out of PSUM, DMA shifts the
                    # values onto the PV lanes, fast reciprocal for A's half
                    # (custom-DVE ops misread PSUM base 64, so A's sums are
                    # copied out and reciprocated after the shift)
                    rec = rc_pool.tile([128, QBLK], F32, tag="rec")
                    rsh = rc_pool.tile([128, QBLK], F32, tag="rsh")
                    rcf = rc_pool.tile([128, QBLK], F32, tag="rcf")
                    nc.vector.tensor_copy(
                        out=rec[64:128, :], in_=at_bA[64:128, :]
                    )
                    nc.vector.reciprocal_approx_fast(
                        out=rec[0:64, :], in_=at_bB[0:64, :]
                    )
                    nc.sync.dma_start(out=rsh[0:64, :], in_=rec[64:128, :])
                    nc.sync.dma_start(out=rsh[64:128, :], in_=rec[0:64, :])
                    nc.vector.reciprocal_approx_fast(
                        out=rcf[0:64, :], in_=rsh[0:64, :]
                    )
                    nc.vector.tensor_mul(
                        out=a2[pair][0:64, :], in0=at_bA[0:64, :], in1=rcf[0:64, :]
                    )
                    nc.vector.tensor_mul(
                        out=a2[pair][64:128, :], in0=at_bB[64:128, :],
                        in1=rsh[64:128, :],
                    )

                pending = (qi, a2)
            emit_outproj(*pending, (0, 1, 2, 3))

    return nc


def _shard_inputs(x, w_qkv, w_proj):
    import ml_dtypes

    bf16 = ml_dtypes.bfloat16
    x = np.ascontiguousarray(np.asarray(x, dtype=np.float32).astype(bf16))
    w_qkv = np.asarray(w_qkv, dtype=np.float32).astype(bf16)
    w_proj = np.asarray(w_proj, dtype=np.float32).astype(bf16)
    # x_all layout: [nb, partition, ct*512 + q] (nb-major column blocks)
    xT = []
    for b in range(B):
        t = np.ascontiguousarray(x[b].T).reshape(8, 128, NQB, QBLK)
        xT.append(
            np.ascontiguousarray(t.transpose(2, 1, 0, 3)).reshape(NQB, 128, 8 * QBLK)
        )
    in_maps = []
    for c in range(NCORES):
        b, g = divmod(c, 4)
        r0 = 64 * HPC * g  # 256 * g
        wq = w_qkv[r0 : r0 + CSL, :]
        wk = w_qkv[C + r0 : C + r0 + CSL, :]
        wvs = w_qkv[2 * C + r0 : 2 * C + r0 + CSL, :]
        wqkT = np.concatenate([wq, wk], axis=0).T.reshape(8, 128, QKW)
        wqkT = np.ascontiguousarray(wqkT.transpose(1, 0, 2)).reshape(1, 128, 8 * QKW)
        wvT = wvs.T.reshape(8, 128, CSL)
        wvT = np.ascontiguousarray(wvT.transpose(1, 0, 2)).reshape(1, 128, 8 * CSL)
        wpT = w_proj[:, r0 : r0 + CSL].T.reshape(2, 128, C)
        wpT = np.ascontiguousarray(wpT.transpose(1, 0, 2)).reshape(1, 128, 2 * C)
        in_maps.append({"xT": xT[b], "wqkT": wqkT, "wvT": wvT, "wpT": wpT})
    return in_maps


def run(x, w_qkv, w_proj, b_proj, trace=False, **spmd_kwargs):
    from concourse.bass_utils import run_bass_kernel_spmd

    in_maps = _shard_inputs(x, w_qkv, w_proj)
    nc = build_nc()
    nc.finalize()
    res = run_bass_kernel_spmd(
        nc, in_maps, core_ids=list(range(NCORES)), trace=trace, **spmd_kwargs
    )
    y = np.zeros((B, N, C), np.float32)
    for c in range(NCORES):
        y[c // 4] += res.results[c]["y"]
    y += np.asarray(b_proj, dtype=np.float32)[None, None, :]
    return y, res


def kernel(x, w_qkv, w_proj, b_proj):
    y, _ = run(x, w_qkv, w_proj, b_proj, trace=False)
    return y
